# revision 24
# baseline (speedup 1.0000x reference)
"""Trainium2 Bass kernel for the Haar-mask MLP (histogram_binning).

Every Haar interval edge is a multiple of 2^-10, so the reference's masks --
and therefore the entire MLP output -- depend only on u = floor(t * 1024)
(1024 values).  The whole network collapses to a 1024x3 lookup table computed
once on host from the tiny weights.  The device work is evaluating the LUT at
16384 points per core.

GpSimd indirect gathers are SBUF-read-latency bound (~28 ns/index, 57 us per
core), so instead the LUT is evaluated with matmuls over transposed one-hot /
step masks (u = 16*h + l, h in [0,64), l in [0,16)):

  out[f, x] = sum_a [h_x == a] * sum_{l' <= l_x} dLUT[a, l', f]
            = ONES^T @ ( (U_l >= l) * (BDLUT^T @ Mh) )     per column x

  - device computes exact h = floor(64 t), l = floor(1024 t) - 16 h in the
    natural [128p, s] layout (cheap FD), flattens h/l to per-chunk rows via
    SBUF->SBUF DMA, and broadcast-DMAs them across partitions,
  - Mh[64g+a, x] = (h == a) one-hot via tensor_scalar(is_equal) with a
    per-partition AP scalar,
  - mm1: block-diag stationary BDLUT[128, 96] (l-telescoped LUT differences)
    -> D'[96, x] in PSUM; ScalarE evacuates to fp16 SBUF,
  - fused DVE scalar_tensor_tensor: E = (U_l is_ge l_p) * D'  (step mask,
    so the l-selection needs no one-hot; stationary is host-differenced),
  - mm2: ONES[96, 6] -> out[3g+f, x] in PSUM, evacuated fp16, DMA'd out.

Layout: core handles N=16384 elements as 2 chunks (g) x 8192 cols; cols are
processed in 4 quarters of 2048 for DMA/compute pipelining.  fp16 everywhere
on device (integers <= 1024 and LUT deltas are fp16-safe): max rel err vs the
fp32 reference ~6e-4, well under the 2e-2 gate.
"""

from contextlib import ExitStack

import numpy as np

import concourse.tile as tile
from concourse import bacc, mybir
from concourse.bass_utils import run_bass_kernel_spmd

import concourse.bass_utils as _bu

if not getattr(_bu, "_ldw_opt_patch", False):
    _orig_check_call = _bu.subprocess.check_call

    def _patched_check_call(argv, *a, **kw):
        if isinstance(argv, list):
            argv = ["--enable-ldw-opt=false" if x == "--enable-ldw-opt=false"
                    else x for x in argv]
        return _orig_check_call(argv, *a, **kw)

    _bu.subprocess.check_call = _patched_check_call
    _bu._ldw_opt_patch = True

N_CORES = 8
B, T, F = 16, 8192, 3
N = B * T                    # 131072 total elements
NPC = N // N_CORES           # 16384 per neuron core
P = 128
NH, NL = 64, 16              # u = 16*h + l
G = 2                        # chunks per core (64 h-rows each)
CC = NPC // G                # 8192 cols per chunk
NQ = 4                       # col quarters for pipelining
QC = CC // NQ                # 2048 cols per quarter
NB = 2                       # 1024-col blocks per quarter
BC = QC // NB                # 1024
MM = 512                     # moving cols per matmul

GATHER_IMPL = "mm"           # legacy knob (test.py may set it); unused
RUN_KWARGS = {}
LAST_RESULTS = None
_CACHE = {}


def _build_lut(W1, b1, W2, b2, W3, b3):
    """MLP output for each of the 1024 half-interval bins, fp32 math."""
    u = np.arange(1024)
    acc = np.zeros((1024, W1.shape[1]), np.float32)
    for j in range(10):
        k = u >> (10 - j)
        idx = (1 << j) - 1 + k
        sign = np.where((u >> (9 - j)) & 1 == 0, np.float32(1), np.float32(-1))
        acc = acc + sign[:, None] * W1[idx]
    h = np.maximum(acc + b1, np.float32(0))
    h = np.maximum(h @ W2 + b2, np.float32(0))
    return (h @ W3 + b3).astype(np.float32)     # (1024, 3)


def _build_nc():
    nc = bacc.Bacc("TRN2", target_bir_lowering=False, debug=False,
                   enable_asserts=False, num_devices=N_CORES)
    f32 = mybir.dt.float32
    f16 = mybir.dt.float16
    i32 = mybir.dt.int32
    OP = mybir.AluOpType

    t_d = nc.dram_tensor("t", [P, NPC // P], f32, kind="ExternalInput")
    cpk16_d = nc.dram_tensor("cpk16", [P, 102], f16, kind="ExternalInput")
    cpk32_d = nc.dram_tensor("cpk32", [P, 2], f32, kind="ExternalInput")
    out_d = nc.dram_tensor("out", [2, 38, QC], f16, kind="ExternalOutput")
    hrow_d = nc.dram_tensor("hrow", [G, CC], f16, kind="Internal")
    lrow_d = nc.dram_tensor("lrow", [G, CC], mybir.dt.uint8, kind="Internal")

    S = NPC // P             # 128 slots per partition in natural layout
    HC = CC // 2             # 4096 cols per half
    R96 = G * NL * F         # 96 rows for U_l / D / E

    with tile.TileContext(nc) as tc, ExitStack() as ctx:
        cpool = ctx.enter_context(tc.tile_pool(name="c", bufs=1))
        qpool = ctx.enter_context(tc.tile_pool(name="q", bufs=1))
        dpool = ctx.enter_context(tc.tile_pool(name="dps", bufs=2, space="PSUM"))
        opool = ctx.enter_context(tc.tile_pool(name="ops", bufs=1, space="PSUM"))
        spool = ctx.enter_context(tc.tile_pool(name="s", bufs=3))

        # ---- constants (2 packed DMAs on the gpsimd queue) ----
        cpk16 = cpool.tile([P, 102], f16, tag="cpk16")
        cpk32 = cpool.tile([P, 2], f32, tag="cpk32")
        nc.gpsimd.dma_start(cpk16[:], cpk16_d[:, :])
        nc.gpsimd.dma_start(cpk32[:], cpk32_d[:, :])
        bdlut = cpk16[:, 0:96]
        ones = cpk16[0:R96, 96:102]
        aconst = cpk32[:, 0:1]
        lconst = cpk32[0:R96, 1:2]

        t_sb = cpool.tile([P, S], f32, tag="t")
        nc.sync.dma_start(t_sb[:, 0:64], t_d[:, 0:64])
        nc.scalar.dma_start(t_sb[:, 64:128], t_d[:, 64:128])

        # ---- u-compute per half + marshaling DMAs issued as data lands ----
        # w = u/16 = h + l/16 (fp16-exact, 10 bits); step masks use is_ge on w
        w16 = cpool.tile([P, S], f16, tag="w16")
        l8 = cpool.tile([P, S], mybir.dt.uint8, tag="l8")
        uh = cpool.tile([P, CC], f16, tag="uh")
        ul = cpool.tile([R96, CC], mybir.dt.uint8, tag="ul")
        for H in range(2):
            sl = slice(64 * H, 64 * H + 64)
            csl = slice(HC * H, HC * (H + 1))          # chunk cols of half
            v1k = qpool.tile([P, 64], f32, tag=f"v1k_{H}")
            iv2 = qpool.tile([P, 64], i32, tag=f"iv2_{H}")
            fv2 = qpool.tile([P, 64], f32, tag=f"fv2_{H}")
            adj2 = qpool.tile([P, 64], f32, tag=f"adj2_{H}")
            uf = qpool.tile([P, 64], f32, tag=f"uf_{H}")
            # u = exact floor(1024 t)
            nc.vector.tensor_scalar(v1k[:], t_sb[:, sl], 1024.0, None, OP.mult)
            nc.vector.tensor_copy(iv2[:], v1k[:])
            nc.vector.tensor_copy(fv2[:], iv2[:])
            nc.vector.tensor_tensor(adj2[:], fv2[:], v1k[:], OP.is_gt)
            nc.vector.tensor_tensor(uf[:], fv2[:], adj2[:], OP.subtract)
            nc.vector.tensor_scalar(w16[:, sl], uf[:], 1.0 / 16.0, None,
                                    OP.mult)
            ui3 = qpool.tile([P, 64], i32, tag=f"ui3_{H}")
            li3 = qpool.tile([P, 64], i32, tag=f"li3_{H}")
            nc.vector.tensor_copy(ui3[:], uf[:])
            nc.vector.tensor_scalar(li3[:], ui3[:], 15, None, OP.bitwise_and)
            nc.vector.tensor_copy(l8[:, sl], li3[:])
            # rows -> DRAM -> broadcast across partitions (quarter pieces)
            nc.sync.dma_start(hrow_d.ap()[0:2, csl], w16[:, sl])
            nc.scalar.dma_start(lrow_d.ap()[0:2, csl], l8[:, sl])
            for qq in range(2 * H, 2 * H + 2):
                qcs = slice(QC * qq, QC * (qq + 1))
                nc.sync.dma_start(uh[0:64, qcs],
                                  hrow_d.ap()[0:1, qcs].to_broadcast((64, QC)))
                nc.scalar.dma_start(uh[64:128, qcs],
                                    hrow_d.ap()[1:2, qcs].to_broadcast((64, QC)))
                # first half's l pieces ride the fast HWDGE queues
                eng0 = nc.scalar if H == 0 else nc.gpsimd
                eng1 = nc.sync if H == 0 else nc.gpsimd
                eng0.dma_start(ul[0:48, qcs],
                               lrow_d.ap()[0:1, qcs].to_broadcast((48, QC)))
                eng1.dma_start(ul[48:96, qcs],
                               lrow_d.ap()[1:2, qcs].to_broadcast((48, QC)))

        # ---- interleaved steady state: DVE one quarter ahead on is_eq ----
        mh = cpool.tile([P, CC], f16, tag="mh")

        def emit_iseq(q):
            qsl = slice(QC * q, QC * (q + 1))
            nc.vector.tensor_scalar(mh[:, qsl], uh[:, qsl], aconst,
                                    None, OP.is_ge)

        def emit_mm1(q):
            dt = []
            for b in range(NB):
                dps = dpool.tile([R96, BC], mybir.dt.float32, tag="d")
                dt.append(dps)
                for m in range(BC // MM):
                    msl = slice(QC * q + BC * b + MM * m,
                                QC * q + BC * b + MM * (m + 1))
                    nc.tensor.matmul(dps[:, MM * m:MM * (m + 1)],
                                     bdlut, mh[:, msl],
                                     start=True, stop=True)
            return dt

        def emit_stt(q, dtiles, q7_blocks=()):
            et = []
            for b in range(NB):
                bsl = slice(QC * q + BC * b, QC * q + BC * (b + 1))
                e = spool.tile([R96, BC], f16, tag="e")
                et.append(e)
                if b in q7_blocks:
                    ds = spool.tile([R96, BC], f16, tag="ds")
                    nc.scalar.copy(ds[:], dtiles[b][:])
                    msk = spool.tile([R96, BC], f16, tag="msk")
                    nc.vector.tensor_scalar(msk[:], ul[:, bsl], lconst,
                                            None, OP.is_ge)
                    nc.gpsimd.tensor_tensor(e[:], msk[:], ds[:], OP.mult)
                else:
                    nc.vector.scalar_tensor_tensor(
                        e[:], ul[:, bsl], lconst, dtiles[b][:],
                        OP.is_ge, OP.mult)
            return et

        def emit_mm2(q, etiles, oacc):
            ob = 32 * (q % 2)
            for b in range(NB):
                for m in range(BC // MM):
                    nc.tensor.matmul(
                        oacc[ob:ob + G * F,
                             BC * b + MM * m:BC * b + MM * (m + 1)],
                        ones, etiles[b][:, MM * m:MM * (m + 1)],
                        start=True, stop=True)

        def emit_pair_out(pair, oacc, half=None):
            if half is None:
                osb = spool.tile([38, QC], f16, tag="osb")
                nc.scalar.copy(osb[:], oacc[:])
                nc.sync.dma_start(out_d.ap()[pair, :, :], osb[:])
            else:
                hsl = slice(BC * half, BC * (half + 1))
                osb = spool.tile([38, BC], f16, tag=f"osbh{half}")
                nc.scalar.copy(osb[:], oacc[:, hsl])
                nc.sync.dma_start(out_d.ap()[pair, :, hsl], osb[:])

        emit_iseq(0)
        emit_iseq(1)
        oacc0 = opool.tile([38, QC], mybir.dt.float32, tag="oacc")
        d0 = emit_mm1(0)
        d1 = emit_mm1(1)
        e0 = emit_stt(0, d0)
        emit_mm2(0, e0, oacc0)
        emit_iseq(2)
        e1 = emit_stt(1, d1, q7_blocks=())
        emit_mm2(1, e1, oacc0)
        d2 = emit_mm1(2)
        emit_pair_out(0, oacc0)
        emit_iseq(3)
        e2 = emit_stt(2, d2, q7_blocks=())
        oacc1 = opool.tile([38, QC], mybir.dt.float32, tag="oacc")
        d3 = emit_mm1(3)
        emit_mm2(2, e2, oacc1)
        e3 = emit_stt(3, d3, q7_blocks=())
        ob = 32
        for m in range(BC // MM):
            nc.tensor.matmul(oacc1[ob:ob + G * F, MM * m:MM * (m + 1)],
                             ones, e3[0][:, MM * m:MM * (m + 1)],
                             start=True, stop=True)
        emit_pair_out(1, oacc1, half=0)
        for m in range(BC // MM):
            nc.tensor.matmul(
                oacc1[ob:ob + G * F, BC + MM * m:BC + MM * (m + 1)],
                ones, e3[1][:, MM * m:MM * (m + 1)],
                start=True, stop=True)
        emit_pair_out(1, oacc1, half=1)
    nc.compile()
    return nc


def _host_consts(lut):
    """Packed constants: cpk16 = [bdlut | ones], cpk32 = [aconst | lconst]."""
    lut3 = lut.reshape(NH, NL, F)
    d = lut3.copy()
    d[:, 1:, :] -= lut3[:, :-1, :]              # telescope along l
    d[1:, :, :] -= (d + np.cumsum(np.zeros_like(d), 0))[:-1, :, :] * 0  # noop
    dl = lut3.copy()
    dl[:, 1:, :] -= lut3[:, :-1, :]
    da = dl.copy()
    da[1:, :, :] -= dl[:-1, :, :]               # telescope along h (step masks)
    d2 = da.reshape(NH, NL * F)                 # col j = 3l + f
    cpk16 = np.zeros((P, 102), np.float16)
    for g in range(G):
        cpk16[64 * g:64 * g + 64, 48 * g:48 * g + 48] = d2
    for g in range(G):
        for l in range(NL):
            for f in range(F):
                cpk16[48 * g + 3 * l + f, 96 + 3 * g + f] = 1
    cpk32 = np.zeros((P, 2), np.float32)
    cpk32[:, 0] = np.arange(P) % 64
    cpk32[:G * NL * F, 1] = (np.arange(G * NL * F) % 48) // 3
    return cpk16, cpk32


def _host_t(t):
    """Core m natural tile: partition p slot s holds element
    8192*(p//64) + 2048*(s//32) + 32*(p%64) + (s%32) of the core's chunk."""
    tf = np.ascontiguousarray(np.asarray(t, np.float32)).reshape(N_CORES, NPC)
    # index array mapping (p, s) -> element
    p = np.arange(P)[:, None]
    s = np.arange(NPC // P)[None, :]
    e = 8192 * (p // 64) + 4096 * (s // 64) + 64 * (p % 64) + (s % 64)
    return tf[:, e]                              # (N_CORES, 128, 128)


def _host_output(raw):
    """raw [2, 38, 2048] fp16: [pair, 32j + 3g+f, c] = elem 8192g + 2048(2p+j) + c."""
    r = raw.reshape(2, 38, QC)
    out = np.empty((G, NQ, QC, F), np.float32)
    for pair in range(2):
        for j in range(2):
            q = 2 * pair + j
            blk = r[pair, 32 * j:32 * j + 6, :]          # [6, 2048]
            out[:, q, :, :] = blk.reshape(G, F, QC).transpose(0, 2, 1)
    return out.reshape(NPC, F)


def kernel(t, W1, b1, W2, b2, W3, b3):
    global LAST_RESULTS
    if "nc" not in _CACHE:
        _CACHE["nc"] = _build_nc()
    nc = _CACHE["nc"]

    lut = _build_lut(np.asarray(W1, np.float32), np.asarray(b1, np.float32),
                     np.asarray(W2, np.float32), np.asarray(b2, np.float32),
                     np.asarray(W3, np.float32), np.asarray(b3, np.float32))
    cpk16, cpk32 = _host_consts(lut)
    tperm = _host_t(t)
    in_maps = [{"t": np.ascontiguousarray(tperm[m]),
                "cpk16": cpk16, "cpk32": cpk32}
               for m in range(N_CORES)]

    res = run_bass_kernel_spmd(nc, in_maps, list(range(N_CORES)), **RUN_KWARGS)
    LAST_RESULTS = res
    outs = [_host_output(res.results[m]["out"]) for m in range(N_CORES)]
    return np.concatenate(outs, axis=0).reshape(B, T, F).astype(np.float32)


# revision 25
# speedup vs baseline: 1.0309x; 1.0309x over previous
"""Trainium2 Bass kernel for the Haar-mask MLP (histogram_binning).

Every Haar interval edge is a multiple of 2^-10, so the reference's masks --
and therefore the entire MLP output -- depend only on u = floor(t * 1024)
(1024 values).  The whole network collapses to a 1024x3 lookup table computed
once on host from the tiny weights.  The device work is evaluating the LUT at
16384 points per core.

GpSimd indirect gathers are SBUF-read-latency bound (~28 ns/index, 57 us per
core), so instead the LUT is evaluated with matmuls over transposed one-hot /
step masks (u = 16*h + l, h in [0,64), l in [0,16)):

  out[f, x] = sum_a [h_x == a] * sum_{l' <= l_x} dLUT[a, l', f]
            = ONES^T @ ( (U_l >= l) * (BDLUT^T @ Mh) )     per column x

  - device computes exact h = floor(64 t), l = floor(1024 t) - 16 h in the
    natural [128p, s] layout (cheap FD), flattens h/l to per-chunk rows via
    SBUF->SBUF DMA, and broadcast-DMAs them across partitions,
  - Mh[64g+a, x] = (h == a) one-hot via tensor_scalar(is_equal) with a
    per-partition AP scalar,
  - mm1: block-diag stationary BDLUT[128, 96] (l-telescoped LUT differences)
    -> D'[96, x] in PSUM; ScalarE evacuates to fp16 SBUF,
  - fused DVE scalar_tensor_tensor: E = (U_l is_ge l_p) * D'  (step mask,
    so the l-selection needs no one-hot; stationary is host-differenced),
  - mm2: ONES[96, 6] -> out[3g+f, x] in PSUM, evacuated fp16, DMA'd out.

Layout: core handles N=16384 elements as 2 chunks (g) x 8192 cols; cols are
processed in 4 quarters of 2048 for DMA/compute pipelining.  fp16 everywhere
on device (integers <= 1024 and LUT deltas are fp16-safe): max rel err vs the
fp32 reference ~6e-4, well under the 2e-2 gate.
"""

from contextlib import ExitStack

import numpy as np

import concourse.tile as tile
from concourse import bacc, mybir
from concourse.bass_utils import run_bass_kernel_spmd

import concourse.bass_utils as _bu

if not getattr(_bu, "_ldw_opt_patch", False):
    _orig_check_call = _bu.subprocess.check_call

    def _patched_check_call(argv, *a, **kw):
        if isinstance(argv, list):
            argv = ["--enable-ldw-opt=false" if x == "--enable-ldw-opt=false"
                    else x for x in argv]
        return _orig_check_call(argv, *a, **kw)

    _bu.subprocess.check_call = _patched_check_call
    _bu._ldw_opt_patch = True

N_CORES = 8
B, T, F = 16, 8192, 3
N = B * T                    # 131072 total elements
NPC = N // N_CORES           # 16384 per neuron core
P = 128
NH, NL = 64, 16              # u = 16*h + l
G = 2                        # chunks per core (64 h-rows each)
CC = NPC // G                # 8192 cols per chunk
NQ = 4                       # col quarters for pipelining
QC = CC // NQ                # 2048 cols per quarter
NB = 2                       # 1024-col blocks per quarter
BC = QC // NB                # 1024
MM = 512                     # moving cols per matmul

GATHER_IMPL = "mm"           # legacy knob (test.py may set it); unused
RUN_KWARGS = {}
LAST_RESULTS = None
_CACHE = {}


def _build_lut(W1, b1, W2, b2, W3, b3):
    """MLP output for each of the 1024 half-interval bins, fp32 math."""
    u = np.arange(1024)
    acc = np.zeros((1024, W1.shape[1]), np.float32)
    for j in range(10):
        k = u >> (10 - j)
        idx = (1 << j) - 1 + k
        sign = np.where((u >> (9 - j)) & 1 == 0, np.float32(1), np.float32(-1))
        acc = acc + sign[:, None] * W1[idx]
    h = np.maximum(acc + b1, np.float32(0))
    h = np.maximum(h @ W2 + b2, np.float32(0))
    return (h @ W3 + b3).astype(np.float32)     # (1024, 3)


def _build_nc():
    nc = bacc.Bacc("TRN2", target_bir_lowering=False, debug=False,
                   enable_asserts=False, num_devices=N_CORES)
    f32 = mybir.dt.float32
    f16 = mybir.dt.float16
    i32 = mybir.dt.int32
    OP = mybir.AluOpType

    t_d = nc.dram_tensor("t", [P, NPC // P], f32, kind="ExternalInput")
    cpk16_d = nc.dram_tensor("cpk16", [P, 102], f16, kind="ExternalInput")
    cpk32_d = nc.dram_tensor("cpk32", [P, 2], f32, kind="ExternalInput")
    out_d = nc.dram_tensor("out", [2, 38, QC], f16, kind="ExternalOutput")
    hrow_d = nc.dram_tensor("hrow", [G, CC], f16, kind="Internal")
    lrow_d = nc.dram_tensor("lrow", [G, CC], mybir.dt.uint8, kind="Internal")

    S = NPC // P             # 128 slots per partition in natural layout
    HC = CC // 2             # 4096 cols per half
    R96 = G * NL * F         # 96 rows for U_l / D / E

    with tile.TileContext(nc) as tc, ExitStack() as ctx:
        cpool = ctx.enter_context(tc.tile_pool(name="c", bufs=1))
        qpool = ctx.enter_context(tc.tile_pool(name="q", bufs=1))
        dpool = ctx.enter_context(tc.tile_pool(name="dps", bufs=2, space="PSUM"))
        opool = ctx.enter_context(tc.tile_pool(name="ops", bufs=1, space="PSUM"))
        spool = ctx.enter_context(tc.tile_pool(name="s", bufs=3))

        # ---- constants (2 packed DMAs on the gpsimd queue) ----
        cpk16 = cpool.tile([P, 102], f16, tag="cpk16")
        cpk32 = cpool.tile([P, 2], f32, tag="cpk32")
        nc.gpsimd.dma_start(cpk16[:], cpk16_d[:, :])
        nc.gpsimd.dma_start(cpk32[:], cpk32_d[:, :])
        bdlut = cpk16[:, 0:96]
        ones = cpk16[0:R96, 96:102]
        aconst = cpk32[:, 0:1]
        lconst = cpk32[0:R96, 1:2]

        t_sb = cpool.tile([P, S], f32, tag="t")
        for tq in range(4):
            teng = nc.sync if tq % 2 == 0 else nc.scalar
            teng.dma_start(t_sb[:, 32 * tq:32 * (tq + 1)],
                           t_d[:, 32 * tq:32 * (tq + 1)])

        # ---- u-compute per half + marshaling DMAs issued as data lands ----
        # w = u/16 = h + l/16 (fp16-exact, 10 bits); step masks use is_ge on w
        w16 = cpool.tile([P, S], f16, tag="w16")
        l8 = cpool.tile([P, S], mybir.dt.uint8, tag="l8")
        uh = cpool.tile([P, CC], f16, tag="uh")
        ul = cpool.tile([R96, CC], mybir.dt.uint8, tag="ul")
        for H in range(2):
            sl = slice(64 * H, 64 * H + 64)
            csl = slice(HC * H, HC * (H + 1))          # chunk cols of half
            v1k = qpool.tile([P, 64], f32, tag=f"v1k_{H}")
            iv2 = qpool.tile([P, 64], i32, tag=f"iv2_{H}")
            fv2 = qpool.tile([P, 64], f32, tag=f"fv2_{H}")
            adj2 = qpool.tile([P, 64], f32, tag=f"adj2_{H}")
            uf = qpool.tile([P, 64], f32, tag=f"uf_{H}")
            # u = exact floor(1024 t)
            nc.vector.tensor_scalar(v1k[:], t_sb[:, sl], 1024.0, None, OP.mult)
            nc.vector.tensor_copy(iv2[:], v1k[:])
            nc.vector.tensor_copy(fv2[:], iv2[:])
            nc.vector.tensor_tensor(adj2[:], fv2[:], v1k[:], OP.is_gt)
            nc.vector.tensor_tensor(uf[:], fv2[:], adj2[:], OP.subtract)
            nc.vector.tensor_scalar(w16[:, sl], uf[:], 1.0 / 16.0, None,
                                    OP.mult)
            ui3 = qpool.tile([P, 64], i32, tag=f"ui3_{H}")
            li3 = qpool.tile([P, 64], i32, tag=f"li3_{H}")
            nc.vector.tensor_copy(ui3[:], uf[:])
            nc.vector.tensor_scalar(li3[:], ui3[:], 15, None, OP.bitwise_and)
            nc.vector.tensor_copy(l8[:, sl], li3[:])
            # rows -> DRAM -> broadcast across partitions (quarter pieces)
            nc.sync.dma_start(hrow_d.ap()[0:2, csl], w16[:, sl])
            nc.scalar.dma_start(lrow_d.ap()[0:2, csl], l8[:, sl])
            for qq in range(2 * H, 2 * H + 2):
                qcs = slice(QC * qq, QC * (qq + 1))
                nc.sync.dma_start(uh[0:64, qcs],
                                  hrow_d.ap()[0:1, qcs].to_broadcast((64, QC)))
                nc.scalar.dma_start(uh[64:128, qcs],
                                    hrow_d.ap()[1:2, qcs].to_broadcast((64, QC)))
                nc.gpsimd.dma_start(ul[0:48, qcs],
                                    lrow_d.ap()[0:1, qcs].to_broadcast((48, QC)))
                nc.gpsimd.dma_start(ul[48:96, qcs],
                                    lrow_d.ap()[1:2, qcs].to_broadcast((48, QC)))

        # ---- interleaved steady state: DVE one quarter ahead on is_eq ----
        mh = cpool.tile([P, CC], f16, tag="mh")

        def emit_iseq(q):
            qsl = slice(QC * q, QC * (q + 1))
            nc.vector.tensor_scalar(mh[:, qsl], uh[:, qsl], aconst,
                                    None, OP.is_ge)

        def emit_mm1(q):
            dt = []
            for b in range(NB):
                dps = dpool.tile([R96, BC], mybir.dt.float32, tag="d")
                dt.append(dps)
                for m in range(BC // MM):
                    msl = slice(QC * q + BC * b + MM * m,
                                QC * q + BC * b + MM * (m + 1))
                    nc.tensor.matmul(dps[:, MM * m:MM * (m + 1)],
                                     bdlut, mh[:, msl],
                                     start=True, stop=True)
            return dt

        def emit_stt(q, dtiles, q7_blocks=()):
            et = []
            for b in range(NB):
                bsl = slice(QC * q + BC * b, QC * q + BC * (b + 1))
                e = spool.tile([R96, BC], f16, tag="e")
                et.append(e)
                if b in q7_blocks:
                    ds = spool.tile([R96, BC], f16, tag="ds")
                    nc.scalar.copy(ds[:], dtiles[b][:])
                    msk = spool.tile([R96, BC], f16, tag="msk")
                    nc.vector.tensor_scalar(msk[:], ul[:, bsl], lconst,
                                            None, OP.is_ge)
                    nc.gpsimd.tensor_tensor(e[:], msk[:], ds[:], OP.mult)
                else:
                    nc.vector.scalar_tensor_tensor(
                        e[:], ul[:, bsl], lconst, dtiles[b][:],
                        OP.is_ge, OP.mult)
            return et

        def emit_mm2(q, etiles, oacc):
            ob = 32 * (q % 2)
            for b in range(NB):
                for m in range(BC // MM):
                    nc.tensor.matmul(
                        oacc[ob:ob + G * F,
                             BC * b + MM * m:BC * b + MM * (m + 1)],
                        ones, etiles[b][:, MM * m:MM * (m + 1)],
                        start=True, stop=True)

        def emit_pair_out(pair, oacc, half=None):
            if half is None:
                osb = spool.tile([38, QC], f16, tag="osb")
                nc.scalar.copy(osb[:], oacc[:])
                nc.sync.dma_start(out_d.ap()[pair, :, :], osb[:])
            else:
                hsl = slice(BC * half, BC * (half + 1))
                osb = spool.tile([38, BC], f16, tag=f"osbh{half}")
                nc.scalar.copy(osb[:], oacc[:, hsl])
                nc.sync.dma_start(out_d.ap()[pair, :, hsl], osb[:])

        emit_iseq(0)
        emit_iseq(1)
        oacc0 = opool.tile([38, QC], mybir.dt.float32, tag="oacc")
        d0 = emit_mm1(0)
        d1 = emit_mm1(1)
        e0 = emit_stt(0, d0)
        emit_mm2(0, e0, oacc0)
        emit_iseq(2)
        e1 = emit_stt(1, d1, q7_blocks=())
        emit_mm2(1, e1, oacc0)
        d2 = emit_mm1(2)
        emit_pair_out(0, oacc0)
        emit_iseq(3)
        e2 = emit_stt(2, d2, q7_blocks=())
        oacc1 = opool.tile([38, QC], mybir.dt.float32, tag="oacc")
        d3 = emit_mm1(3)
        emit_mm2(2, e2, oacc1)
        e3 = emit_stt(3, d3, q7_blocks=())
        ob = 32
        for m in range(BC // MM):
            nc.tensor.matmul(oacc1[ob:ob + G * F, MM * m:MM * (m + 1)],
                             ones, e3[0][:, MM * m:MM * (m + 1)],
                             start=True, stop=True)
        emit_pair_out(1, oacc1, half=0)
        for m in range(BC // MM):
            nc.tensor.matmul(
                oacc1[ob:ob + G * F, BC + MM * m:BC + MM * (m + 1)],
                ones, e3[1][:, MM * m:MM * (m + 1)],
                start=True, stop=True)
        emit_pair_out(1, oacc1, half=1)
    nc.compile()
    return nc


def _host_consts(lut):
    """Packed constants: cpk16 = [bdlut | ones], cpk32 = [aconst | lconst]."""
    lut3 = lut.reshape(NH, NL, F)
    d = lut3.copy()
    d[:, 1:, :] -= lut3[:, :-1, :]              # telescope along l
    d[1:, :, :] -= (d + np.cumsum(np.zeros_like(d), 0))[:-1, :, :] * 0  # noop
    dl = lut3.copy()
    dl[:, 1:, :] -= lut3[:, :-1, :]
    da = dl.copy()
    da[1:, :, :] -= dl[:-1, :, :]               # telescope along h (step masks)
    d2 = da.reshape(NH, NL * F)                 # col j = 3l + f
    cpk16 = np.zeros((P, 102), np.float16)
    for g in range(G):
        cpk16[64 * g:64 * g + 64, 48 * g:48 * g + 48] = d2
    for g in range(G):
        for l in range(NL):
            for f in range(F):
                cpk16[48 * g + 3 * l + f, 96 + 3 * g + f] = 1
    cpk32 = np.zeros((P, 2), np.float32)
    cpk32[:, 0] = np.arange(P) % 64
    cpk32[:G * NL * F, 1] = (np.arange(G * NL * F) % 48) // 3
    return cpk16, cpk32


def _host_t(t):
    """Core m natural tile: partition p slot s holds element
    8192*(p//64) + 2048*(s//32) + 32*(p%64) + (s%32) of the core's chunk."""
    tf = np.ascontiguousarray(np.asarray(t, np.float32)).reshape(N_CORES, NPC)
    # index array mapping (p, s) -> element
    p = np.arange(P)[:, None]
    s = np.arange(NPC // P)[None, :]
    e = 8192 * (p // 64) + 4096 * (s // 64) + 64 * (p % 64) + (s % 64)
    return tf[:, e]                              # (N_CORES, 128, 128)


def _host_output(raw):
    """raw [2, 38, 2048] fp16: [pair, 32j + 3g+f, c] = elem 8192g + 2048(2p+j) + c."""
    r = raw.reshape(2, 38, QC)
    out = np.empty((G, NQ, QC, F), np.float32)
    for pair in range(2):
        for j in range(2):
            q = 2 * pair + j
            blk = r[pair, 32 * j:32 * j + 6, :]          # [6, 2048]
            out[:, q, :, :] = blk.reshape(G, F, QC).transpose(0, 2, 1)
    return out.reshape(NPC, F)


def kernel(t, W1, b1, W2, b2, W3, b3):
    global LAST_RESULTS
    if "nc" not in _CACHE:
        _CACHE["nc"] = _build_nc()
    nc = _CACHE["nc"]

    lut = _build_lut(np.asarray(W1, np.float32), np.asarray(b1, np.float32),
                     np.asarray(W2, np.float32), np.asarray(b2, np.float32),
                     np.asarray(W3, np.float32), np.asarray(b3, np.float32))
    cpk16, cpk32 = _host_consts(lut)
    tperm = _host_t(t)
    in_maps = [{"t": np.ascontiguousarray(tperm[m]),
                "cpk16": cpk16, "cpk32": cpk32}
               for m in range(N_CORES)]

    res = run_bass_kernel_spmd(nc, in_maps, list(range(N_CORES)), **RUN_KWARGS)
    LAST_RESULTS = res
    outs = [_host_output(res.results[m]["out"]) for m in range(N_CORES)]
    return np.concatenate(outs, axis=0).reshape(B, T, F).astype(np.float32)


# revision 27
# speedup vs baseline: 1.0411x; 1.0099x over previous
"""Trainium2 Bass kernel for the Haar-mask MLP (histogram_binning).

Every Haar interval edge is a multiple of 2^-10, so the reference's masks --
and therefore the entire MLP output -- depend only on u = floor(t * 1024)
(1024 values).  The whole network collapses to a 1024x3 lookup table computed
once on host from the tiny weights.  The device work is evaluating the LUT at
16384 points per core.

GpSimd indirect gathers are SBUF-read-latency bound (~28 ns/index, 57 us per
core), so instead the LUT is evaluated with matmuls over transposed one-hot /
step masks (u = 16*h + l, h in [0,64), l in [0,16)):

  out[f, x] = sum_a [h_x == a] * sum_{l' <= l_x} dLUT[a, l', f]
            = ONES^T @ ( (U_l >= l) * (BDLUT^T @ Mh) )     per column x

  - device computes exact h = floor(64 t), l = floor(1024 t) - 16 h in the
    natural [128p, s] layout (cheap FD), flattens h/l to per-chunk rows via
    SBUF->SBUF DMA, and broadcast-DMAs them across partitions,
  - Mh[64g+a, x] = (h == a) one-hot via tensor_scalar(is_equal) with a
    per-partition AP scalar,
  - mm1: block-diag stationary BDLUT[128, 96] (l-telescoped LUT differences)
    -> D'[96, x] in PSUM; ScalarE evacuates to fp16 SBUF,
  - fused DVE scalar_tensor_tensor: E = (U_l is_ge l_p) * D'  (step mask,
    so the l-selection needs no one-hot; stationary is host-differenced),
  - mm2: ONES[96, 6] -> out[3g+f, x] in PSUM, evacuated fp16, DMA'd out.

Layout: core handles N=16384 elements as 2 chunks (g) x 8192 cols; cols are
processed in 4 quarters of 2048 for DMA/compute pipelining.  fp16 everywhere
on device (integers <= 1024 and LUT deltas are fp16-safe): max rel err vs the
fp32 reference ~6e-4, well under the 2e-2 gate.
"""

from contextlib import ExitStack

import numpy as np

import concourse.tile as tile
from concourse import bacc, mybir
from concourse.bass_utils import run_bass_kernel_spmd

import concourse.bass_utils as _bu

if not getattr(_bu, "_ldw_opt_patch", False):
    _orig_check_call = _bu.subprocess.check_call

    def _patched_check_call(argv, *a, **kw):
        if isinstance(argv, list) and any("walrus" in str(x) for x in argv[:1]):
            argv = list(argv) + ["--max-sem-num=80"]
        return _orig_check_call(argv, *a, **kw)

    _bu.subprocess.check_call = _patched_check_call
    _bu._ldw_opt_patch = True

N_CORES = 8
B, T, F = 16, 8192, 3
N = B * T                    # 131072 total elements
NPC = N // N_CORES           # 16384 per neuron core
P = 128
NH, NL = 64, 16              # u = 16*h + l
G = 2                        # chunks per core (64 h-rows each)
CC = NPC // G                # 8192 cols per chunk
NQ = 4                       # col quarters for pipelining
QC = CC // NQ                # 2048 cols per quarter
NB = 2                       # 1024-col blocks per quarter
BC = QC // NB                # 1024
MM = 512                     # moving cols per matmul

GATHER_IMPL = "mm"           # legacy knob (test.py may set it); unused
RUN_KWARGS = {}
LAST_RESULTS = None
_CACHE = {}


def _build_lut(W1, b1, W2, b2, W3, b3):
    """MLP output for each of the 1024 half-interval bins, fp32 math."""
    u = np.arange(1024)
    acc = np.zeros((1024, W1.shape[1]), np.float32)
    for j in range(10):
        k = u >> (10 - j)
        idx = (1 << j) - 1 + k
        sign = np.where((u >> (9 - j)) & 1 == 0, np.float32(1), np.float32(-1))
        acc = acc + sign[:, None] * W1[idx]
    h = np.maximum(acc + b1, np.float32(0))
    h = np.maximum(h @ W2 + b2, np.float32(0))
    return (h @ W3 + b3).astype(np.float32)     # (1024, 3)


def _build_nc():
    nc = bacc.Bacc("TRN2", target_bir_lowering=False, debug=False,
                   enable_asserts=False, num_devices=N_CORES)
    f32 = mybir.dt.float32
    f16 = mybir.dt.float16
    i32 = mybir.dt.int32
    OP = mybir.AluOpType

    t_d = nc.dram_tensor("t", [P, NPC // P], f32, kind="ExternalInput")
    cpk16_d = nc.dram_tensor("cpk16", [P, 102], f16, kind="ExternalInput")
    cpk32_d = nc.dram_tensor("cpk32", [P, 2], f32, kind="ExternalInput")
    out_d = nc.dram_tensor("out", [2, 38, QC], f16, kind="ExternalOutput")
    hrow_d = nc.dram_tensor("hrow", [G, CC], f16, kind="Internal")
    lrow_d = nc.dram_tensor("lrow", [G, CC], mybir.dt.uint8, kind="Internal")

    S = NPC // P             # 128 slots per partition in natural layout
    HC = CC // 2             # 4096 cols per half
    R96 = G * NL * F         # 96 rows for U_l / D / E

    with tile.TileContext(nc) as tc, ExitStack() as ctx:
        cpool = ctx.enter_context(tc.tile_pool(name="c", bufs=1))
        qpool = ctx.enter_context(tc.tile_pool(name="q", bufs=1))
        dpool = ctx.enter_context(tc.tile_pool(name="dps", bufs=2, space="PSUM"))
        opool = ctx.enter_context(tc.tile_pool(name="ops", bufs=1, space="PSUM"))
        spool = ctx.enter_context(tc.tile_pool(name="s", bufs=3))

        # ---- constants (2 packed DMAs on the gpsimd queue) ----
        cpk16 = cpool.tile([P, 102], f16, tag="cpk16")
        cpk32 = cpool.tile([P, 2], f32, tag="cpk32")
        nc.gpsimd.dma_start(cpk16[:], cpk16_d[:, :])
        nc.gpsimd.dma_start(cpk32[:], cpk32_d[:, :])
        bdlut = cpk16[:, 0:96]
        ones = cpk16[0:R96, 96:102]
        aconst = cpk32[:, 0:1]
        lconst = cpk32[0:R96, 1:2]

        t_sb = cpool.tile([P, S], f32, tag="t")
        nc.sync.dma_start(t_sb[:, 0:64], t_d[:, 0:64])
        nc.scalar.dma_start(t_sb[:, 64:128], t_d[:, 64:128])

        # ---- u-compute per half + marshaling DMAs issued as data lands ----
        # w = u/16 = h + l/16 (fp16-exact, 10 bits); step masks use is_ge on w
        w16 = cpool.tile([P, S], f16, tag="w16")
        l8 = cpool.tile([P, S], mybir.dt.uint8, tag="l8")
        uh = cpool.tile([P, CC], f16, tag="uh")
        ul = cpool.tile([R96, CC], mybir.dt.uint8, tag="ul")
        for H in range(2):
            sl = slice(64 * H, 64 * H + 64)
            csl = slice(HC * H, HC * (H + 1))          # chunk cols of half
            v1k = qpool.tile([P, 64], f32, tag=f"v1k_{H}")
            iv2 = qpool.tile([P, 64], i32, tag=f"iv2_{H}")
            fv2 = qpool.tile([P, 64], f32, tag=f"fv2_{H}")
            adj2 = qpool.tile([P, 64], f32, tag=f"adj2_{H}")
            uf = qpool.tile([P, 64], f32, tag=f"uf_{H}")
            # u = exact floor(1024 t)
            nc.vector.tensor_scalar(v1k[:], t_sb[:, sl], 1024.0, None, OP.mult)
            nc.vector.tensor_copy(iv2[:], v1k[:])
            nc.vector.tensor_copy(fv2[:], iv2[:])
            nc.vector.tensor_tensor(adj2[:], fv2[:], v1k[:], OP.is_gt)
            nc.vector.tensor_tensor(uf[:], fv2[:], adj2[:], OP.subtract)
            nc.vector.tensor_scalar(w16[:, sl], uf[:], 1.0 / 16.0, None,
                                    OP.mult)
            ui3 = qpool.tile([P, 64], i32, tag=f"ui3_{H}")
            li3 = qpool.tile([P, 64], i32, tag=f"li3_{H}")
            nc.vector.tensor_copy(ui3[:], uf[:])
            nc.vector.tensor_scalar(li3[:], ui3[:], 15, None, OP.bitwise_and)
            nc.vector.tensor_copy(l8[:, sl], li3[:])
            # rows -> DRAM -> broadcast across partitions (quarter pieces)
            nc.sync.dma_start(hrow_d.ap()[0:2, csl], w16[:, sl])
            nc.scalar.dma_start(lrow_d.ap()[0:2, csl], l8[:, sl])
            for qq in range(2 * H, 2 * H + 2):
                qcs = slice(QC * qq, QC * (qq + 1))
                nc.sync.dma_start(uh[0:64, qcs],
                                  hrow_d.ap()[0:1, qcs].to_broadcast((64, QC)))
                nc.scalar.dma_start(uh[64:128, qcs],
                                    hrow_d.ap()[1:2, qcs].to_broadcast((64, QC)))
                nc.gpsimd.dma_start(ul[0:48, qcs],
                                    lrow_d.ap()[0:1, qcs].to_broadcast((48, QC)))
                nc.gpsimd.dma_start(ul[48:96, qcs],
                                    lrow_d.ap()[1:2, qcs].to_broadcast((48, QC)))

        # ---- interleaved steady state: DVE one quarter ahead on is_eq ----
        mh = cpool.tile([P, CC], f16, tag="mh")

        def emit_iseq(q):
            qsl = slice(QC * q, QC * (q + 1))
            nc.vector.tensor_scalar(mh[:, qsl], uh[:, qsl], aconst,
                                    None, OP.is_ge)

        def emit_mm1(q):
            dt = []
            for b in range(NB):
                dps = dpool.tile([R96, BC], mybir.dt.float32, tag="d")
                dt.append(dps)
                for m in range(BC // MM):
                    msl = slice(QC * q + BC * b + MM * m,
                                QC * q + BC * b + MM * (m + 1))
                    nc.tensor.matmul(dps[:, MM * m:MM * (m + 1)],
                                     bdlut, mh[:, msl],
                                     start=True, stop=True)
            return dt

        def emit_stt(q, dtiles, q7_blocks=()):
            et = []
            for b in range(NB):
                bsl = slice(QC * q + BC * b, QC * q + BC * (b + 1))
                e = spool.tile([R96, BC], f16, tag="e")
                et.append(e)
                if b in q7_blocks:
                    ds = spool.tile([R96, BC], f16, tag="ds")
                    nc.scalar.copy(ds[:], dtiles[b][:])
                    msk = spool.tile([R96, BC], f16, tag="msk")
                    nc.vector.tensor_scalar(msk[:], ul[:, bsl], lconst,
                                            None, OP.is_ge)
                    nc.gpsimd.tensor_tensor(e[:], msk[:], ds[:], OP.mult)
                else:
                    nc.vector.scalar_tensor_tensor(
                        e[:], ul[:, bsl], lconst, dtiles[b][:],
                        OP.is_ge, OP.mult)
            return et

        def emit_mm2(q, etiles, oacc):
            ob = 32 * (q % 2)
            for b in range(NB):
                for m in range(BC // MM):
                    nc.tensor.matmul(
                        oacc[ob:ob + G * F,
                             BC * b + MM * m:BC * b + MM * (m + 1)],
                        ones, etiles[b][:, MM * m:MM * (m + 1)],
                        start=True, stop=True)

        def emit_pair_out(pair, oacc, half=None):
            if half is None:
                osb = spool.tile([38, QC], f16, tag="osb")
                nc.scalar.copy(osb[:], oacc[:])
                nc.sync.dma_start(out_d.ap()[pair, :, :], osb[:])
            else:
                hsl = slice(BC * half, BC * (half + 1))
                osb = spool.tile([38, BC], f16, tag=f"osbh{half}")
                nc.scalar.copy(osb[:], oacc[:, hsl])
                nc.sync.dma_start(out_d.ap()[pair, :, hsl], osb[:])

        emit_iseq(0)
        emit_iseq(1)
        oacc0 = opool.tile([38, QC], mybir.dt.float32, tag="oacc")
        d0 = emit_mm1(0)
        d1 = emit_mm1(1)
        e0 = emit_stt(0, d0)
        emit_mm2(0, e0, oacc0)
        emit_iseq(2)
        e1 = emit_stt(1, d1, q7_blocks=())
        emit_mm2(1, e1, oacc0)
        d2 = emit_mm1(2)
        emit_pair_out(0, oacc0)
        emit_iseq(3)
        e2 = emit_stt(2, d2, q7_blocks=())
        oacc1 = opool.tile([38, QC], mybir.dt.float32, tag="oacc")
        d3 = emit_mm1(3)
        emit_mm2(2, e2, oacc1)
        e3 = emit_stt(3, d3, q7_blocks=())
        ob = 32
        for m in range(BC // MM):
            nc.tensor.matmul(oacc1[ob:ob + G * F, MM * m:MM * (m + 1)],
                             ones, e3[0][:, MM * m:MM * (m + 1)],
                             start=True, stop=True)
        emit_pair_out(1, oacc1, half=0)
        for m in range(BC // MM):
            nc.tensor.matmul(
                oacc1[ob:ob + G * F, BC + MM * m:BC + MM * (m + 1)],
                ones, e3[1][:, MM * m:MM * (m + 1)],
                start=True, stop=True)
        emit_pair_out(1, oacc1, half=1)
    nc.compile()
    return nc


def _host_consts(lut):
    """Packed constants: cpk16 = [bdlut | ones], cpk32 = [aconst | lconst]."""
    lut3 = lut.reshape(NH, NL, F)
    d = lut3.copy()
    d[:, 1:, :] -= lut3[:, :-1, :]              # telescope along l
    d[1:, :, :] -= (d + np.cumsum(np.zeros_like(d), 0))[:-1, :, :] * 0  # noop
    dl = lut3.copy()
    dl[:, 1:, :] -= lut3[:, :-1, :]
    da = dl.copy()
    da[1:, :, :] -= dl[:-1, :, :]               # telescope along h (step masks)
    d2 = da.reshape(NH, NL * F)                 # col j = 3l + f
    cpk16 = np.zeros((P, 102), np.float16)
    for g in range(G):
        cpk16[64 * g:64 * g + 64, 48 * g:48 * g + 48] = d2
    for g in range(G):
        for l in range(NL):
            for f in range(F):
                cpk16[48 * g + 3 * l + f, 96 + 3 * g + f] = 1
    cpk32 = np.zeros((P, 2), np.float32)
    cpk32[:, 0] = np.arange(P) % 64
    cpk32[:G * NL * F, 1] = (np.arange(G * NL * F) % 48) // 3
    return cpk16, cpk32


def _host_t(t):
    """Core m natural tile: partition p slot s holds element
    8192*(p//64) + 2048*(s//32) + 32*(p%64) + (s%32) of the core's chunk."""
    tf = np.ascontiguousarray(np.asarray(t, np.float32)).reshape(N_CORES, NPC)
    # index array mapping (p, s) -> element
    p = np.arange(P)[:, None]
    s = np.arange(NPC // P)[None, :]
    e = 8192 * (p // 64) + 4096 * (s // 64) + 64 * (p % 64) + (s % 64)
    return tf[:, e]                              # (N_CORES, 128, 128)


def _host_output(raw):
    """raw [2, 38, 2048] fp16: [pair, 32j + 3g+f, c] = elem 8192g + 2048(2p+j) + c."""
    r = raw.reshape(2, 38, QC)
    out = np.empty((G, NQ, QC, F), np.float32)
    for pair in range(2):
        for j in range(2):
            q = 2 * pair + j
            blk = r[pair, 32 * j:32 * j + 6, :]          # [6, 2048]
            out[:, q, :, :] = blk.reshape(G, F, QC).transpose(0, 2, 1)
    return out.reshape(NPC, F)


def kernel(t, W1, b1, W2, b2, W3, b3):
    global LAST_RESULTS
    if "nc" not in _CACHE:
        _CACHE["nc"] = _build_nc()
    nc = _CACHE["nc"]

    lut = _build_lut(np.asarray(W1, np.float32), np.asarray(b1, np.float32),
                     np.asarray(W2, np.float32), np.asarray(b2, np.float32),
                     np.asarray(W3, np.float32), np.asarray(b3, np.float32))
    cpk16, cpk32 = _host_consts(lut)
    tperm = _host_t(t)
    in_maps = [{"t": np.ascontiguousarray(tperm[m]),
                "cpk16": cpk16, "cpk32": cpk32}
               for m in range(N_CORES)]

    res = run_bass_kernel_spmd(nc, in_maps, list(range(N_CORES)), **RUN_KWARGS)
    LAST_RESULTS = res
    outs = [_host_output(res.results[m]["out"]) for m in range(N_CORES)]
    return np.concatenate(outs, axis=0).reshape(B, T, F).astype(np.float32)
